# revision 3
# baseline (speedup 1.0000x reference)
"""Trainium2 Bass kernel for nn_Attention_15908558865595.

Math: query [B,H,S] is expanded so qk[b,h,s,:] is constant along the
softmax axis, and jax.nn.softmax subtracts the row max, so the attention
weights are exactly uniform (1/F). The output is therefore
    out[b,h,s,f] = mean(value[b,h,:,0])
broadcast over [S,F] — independent of query/key. The kernel computes the
per-(b,h) mean on device and broadcast-writes the 128 MiB output at HBM
write roofline. Sharding: batch*heads (32 pairs) split 4-per-core across
8 NeuronCores; no cross-device communication.
"""
import sys

if "/opt/trn_rl_repo" not in sys.path:
    sys.path.insert(0, "/opt/trn_rl_repo")

import numpy as np

B, H, S, F = 2, 16, 1024, 1024
N_CORES = 8
BH = B * H
BH_PER_CORE = BH // N_CORES      # 4
P = 128
SLAB = S * F                     # one (b,h) output slab
FILL_COLS = 4096                 # fill tile [128, 4096] f32 = 2 MiB
CHUNK = P * FILL_COLS            # 2 MiB in elements
CHUNKS_PER_SLAB = SLAB // CHUNK  # 2

_NC = None


def _build():
    import concourse.bass as bass
    import concourse.bacc as bacc
    import concourse.tile as tile
    from concourse import mybir

    nc = bacc.Bacc("TRN2", target_bir_lowering=False, debug=False, num_devices=N_CORES)

    v_ap = nc.dram_tensor("v", [BH_PER_CORE, F], mybir.dt.float32, kind="ExternalInput").ap()
    out_ap = nc.dram_tensor(
        "out", [BH_PER_CORE * SLAB], mybir.dt.float32, kind="ExternalOutput"
    ).ap()
    scratch_ap = nc.dram_tensor("scratch", [BH_PER_CORE, 1], mybir.dt.float32).ap()

    with tile.TileContext(nc) as tc:
        with tc.tile_pool(name="small", bufs=1) as small, \
             tc.tile_pool(name="fills", bufs=4) as fills:
            vtile = small.tile([BH_PER_CORE, F], mybir.dt.float32)
            nc.sync.dma_start(vtile[:], v_ap[:])

            vmean = small.tile([BH_PER_CORE, 1], mybir.dt.float32)
            nc.vector.reduce_sum(vmean[:], vtile[:], axis=mybir.AxisListType.X)
            nc.scalar.mul(vmean[:], vmean[:], 1.0 / F)

            # Roundtrip through DRAM broadcasts the 4 per-partition means to
            # all 128 partitions (DMA source with stride-0 partition dim).
            nc.sync.dma_start(scratch_ap[:], vmean[:])
            bc = small.tile([P, BH_PER_CORE], mybir.dt.float32)
            nc.sync.dma_start(
                bc[:], scratch_ap.rearrange("a b -> b a").to_broadcast((P, BH_PER_CORE))
            )

            for i in range(BH_PER_CORE):
                fill = fills.tile([P, FILL_COLS], mybir.dt.float32)
                nc.vector.tensor_copy(
                    out=fill[:], in_=bc[:, i : i + 1].to_broadcast((P, FILL_COLS))
                )
                for j in range(CHUNKS_PER_SLAB):
                    k = i * CHUNKS_PER_SLAB + j
                    dst = out_ap[bass.ts(k, CHUNK)].rearrange("(p x) -> p x", p=P)
                    eng = nc.sync if k % 2 == 0 else nc.scalar
                    eng.dma_start(dst, fill[:])
    nc.compile()
    return nc


def _get_nc():
    global _NC
    if _NC is None:
        _NC = _build()
    return _NC


def run_device(value_flat: np.ndarray, **spmd_kwargs):
    """value_flat: [BH, F] f32. Returns (out [BH, S, F], BassKernelResults)."""
    from concourse.bass_utils import run_bass_kernel_spmd

    nc = _get_nc()
    in_maps = [
        {"v": np.ascontiguousarray(value_flat[c * BH_PER_CORE : (c + 1) * BH_PER_CORE])}
        for c in range(N_CORES)
    ]
    res = run_bass_kernel_spmd(nc, in_maps, list(range(N_CORES)), **spmd_kwargs)
    out = np.empty((BH, S, F), dtype=np.float32)
    for c in range(N_CORES):
        out[c * BH_PER_CORE : (c + 1) * BH_PER_CORE] = res.results[c]["out"].reshape(
            BH_PER_CORE, S, F
        )
    return out, res


def kernel(query: np.ndarray, key: np.ndarray, value: np.ndarray) -> np.ndarray:
    value_flat = np.ascontiguousarray(
        np.asarray(value, dtype=np.float32).reshape(BH, F)
    )
    out, _ = run_device(value_flat)
    return out.reshape(B, H, S, F)
